# revision 22
# baseline (speedup 1.0000x reference)
"""CBAM channel attention kernel for Trainium2 (8 NeuronCores, batch-parallel).

x: [32, 768, 56, 56] f32.  Each core handles 4 samples.  Per (sample,
channel-chunk) we keep the [128, 3136] x-tile resident in SBUF between the
pooling pass and the scale pass, so HBM traffic is 1 read + 1 write of x.

Pooling: mean on ScalarE (activation Copy + accum_out, scale=1/HW folds the
mean), max on VectorE (tensor_reduce).  MLP runs in transposed form on
TensorE: hT = w1T.T @ pooledT, gelu_exact via Erf (keeps ACT in one table
set with Sigmoid), mlpT = (0.5*w2T).T @ hh'.  Gate applied with per-partition
tensor_scalar_mul in place, then DMA out.
"""

import numpy as np

import concourse.bacc as bacc
import concourse.bass as bass
import concourse.mybir as mybir
import concourse.tile as tile
from concourse.bass_utils import run_bass_kernel_spmd

B = 32
C = 768
HW = 56 * 56  # 3136
HID = 48      # C // 16
NCORES = 8
B_LOC = B // NCORES  # 4
KC = C // 128        # 6 channel chunks
F32 = mybir.dt.float32
AF = mybir.ActivationFunctionType
ALU = mybir.AluOpType

_cache = {}


def _build_nc():
    nc = bacc.Bacc("TRN2", target_bir_lowering=False, debug=False)
    x_d = nc.declare_dram_parameter("x", [B_LOC * C, HW], F32, isOutput=False)
    # host-pretransposed weights: w1t[p, k, h] = w1[h, k*128+p],
    # w2t[h, k, p] = 0.5 * w2[k*128+p, h]  (0.5 folds the gelu half)
    w1_d = nc.declare_dram_parameter("w1t", [128, KC * HID], F32, isOutput=False)
    w2_d = nc.declare_dram_parameter("w2t", [HID, KC * 128], F32, isOutput=False)
    out_d = nc.declare_dram_parameter("out", [B_LOC * C, HW], F32, isOutput=True)

    with tile.TileContext(nc) as tc:
        with (
            tc.tile_pool(name="consts", bufs=1) as consts,
            tc.tile_pool(name="xtiles", bufs=6) as xpool,
            tc.tile_pool(name="otiles", bufs=9) as opool,
            tc.tile_pool(name="pooled", bufs=3) as pooled_pool,
            tc.tile_pool(name="small", bufs=3) as small_pool,
            tc.tile_pool(name="psum", bufs=2, space="PSUM") as psum_pool,
        ):
            w1T = consts.tile([128, KC, HID], F32)
            nc.sync.dma_start(
                out=w1T, in_=w1_d.rearrange("p (k h) -> p k h", k=KC)
            )
            w2T = consts.tile([HID, KC, 128], F32)
            nc.sync.dma_start(
                out=w2T, in_=w2_d.rearrange("h (k p) -> h k p", k=KC)
            )

            for b in range(B_LOC):
                ots = []
                pooled = pooled_pool.tile([128, KC, 2], F32)
                for k in range(KC):
                    xt = xpool.tile([128, HW], F32, tag="x")
                    ot = opool.tile([128, HW], F32, tag="o")
                    row = (b * KC + k) * 128
                    nc.sync.dma_start(out=xt, in_=x_d[row : row + 128, :])
                    # sum-pool on ACT while staging x into the write buffer;
                    # xt frees right after pooling, decoupled from the gate
                    nc.scalar.activation(
                        out=ot,
                        in_=xt,
                        func=AF.Copy,
                        accum_out=pooled[:, k, 0:1],
                    )
                    nc.vector.reduce_max(
                        out=pooled[:, k, 1:2], in_=xt, axis=mybir.AxisListType.X
                    )
                    ots.append(ot)

                # hT [48, 2] = sum_k w1T_k.T @ pooledT_k
                hps = psum_pool.tile([HID, 2], F32, tag="hps")
                for k in range(KC):
                    nc.tensor.matmul(
                        hps,
                        w1T[:, k, :],
                        pooled[:, k, :],
                        start=(k == 0),
                        stop=(k == KC - 1),
                    )
                # avg column holds the raw sum; scale to the mean here (cheaper
                # than scaling 6 [128,1] pooled slots or a [128,3136] tile)
                nc.vector.tensor_scalar_mul(hps[:, 0:1], hps[:, 0:1], 1.0 / HW)
                e_sb = small_pool.tile([HID, 2], F32, tag="e")
                nc.scalar.activation(
                    out=e_sb, in_=hps, func=AF.Erf, scale=0.7071067811865476
                )
                # hh' = (e + 1) * u   (u = pre-gelu matmul output); the gate
                # path is linear in hh, so accum_out sums avg+max columns
                # directly into hsum for matmul2
                hh = small_pool.tile([HID, 2], F32, tag="hh")
                hsum = small_pool.tile([HID, 1], F32, tag="hsum")
                nc.vector.scalar_tensor_tensor(
                    out=hh, in0=e_sb, scalar=1.0, in1=hps,
                    op0=ALU.add, op1=ALU.mult, accum_out=hsum,
                )
                mlp = psum_pool.tile([128, KC], F32, tag="mlp")
                for k in range(KC):
                    nc.tensor.matmul(
                        mlp[:, k : k + 1],
                        w2T[:, k, :],
                        hsum,
                        start=True,
                        stop=True,
                    )
                gate = small_pool.tile([128, KC], F32, tag="gate")
                nc.scalar.activation(out=gate, in_=mlp, func=AF.Sigmoid)

                for k in range(KC):
                    ot = ots[k]
                    row = (b * KC + k) * 128
                    # gate-multiplies stay off DVE entirely: DVE then runs only
                    # 1x-mode reduces (dedicated port), so GpSimd muls never
                    # contend for the shared 2nd port; ACT takes two to help
                    if k % 3 == 2:
                        nc.scalar.activation(
                            out=ot, in_=ot, func=AF.Copy,
                            scale=gate[:, k : k + 1],
                        )
                    else:
                        nc.gpsimd.tensor_scalar_mul(ot, ot, gate[:, k : k + 1])
                    # writes ride SWDGE (GpSimd) so they never head-of-line
                    # block the read FIFO on the Sync HWDGE ring
                    nc.gpsimd.dma_start(out=out_d[row : row + 128, :], in_=ot)
    nc.finalize()
    return nc


def kernel(x, w1, w2, _trace=False):
    if "nc" not in _cache:
        _cache["nc"] = _build_nc()
    nc = _cache["nc"]

    x = np.ascontiguousarray(x, dtype=np.float32)
    w1t = np.ascontiguousarray(
        np.asarray(w1, np.float32).reshape(HID, KC, 128).transpose(2, 1, 0)
        .reshape(128, KC * HID)
    )
    w2t = np.ascontiguousarray(
        (0.5 * np.asarray(w2, np.float32)).reshape(KC, 128, HID)
        .transpose(2, 0, 1).reshape(HID, KC * 128)
    )
    in_maps = [
        {
            "x": x[i * B_LOC : (i + 1) * B_LOC].reshape(B_LOC * C, HW),
            "w1t": w1t,
            "w2t": w2t,
        }
        for i in range(NCORES)
    ]
    res = run_bass_kernel_spmd(nc, in_maps, core_ids=list(range(NCORES)),
                               trace=_trace)
    out = np.concatenate(
        [r["out"].reshape(B_LOC, C, 56, 56) for r in res.results], axis=0
    )
    if _trace:
        _cache["last_results"] = res
    return out


# revision 23
# speedup vs baseline: 4.0476x; 4.0476x over previous
"""CBAM channel attention kernel for Trainium2 (8 NeuronCores, batch-parallel).

x: [32, 768, 56, 56] f32.  Each core handles 4 samples.  Per (sample,
channel-chunk) we keep the [128, 3136] x-tile resident in SBUF between the
pooling pass and the scale pass, so HBM traffic is 1 read + 1 write of x.

Pooling: mean on ScalarE (activation Copy + accum_out, scale=1/HW folds the
mean), max on VectorE (tensor_reduce).  MLP runs in transposed form on
TensorE: hT = w1T.T @ pooledT, gelu_exact via Erf (keeps ACT in one table
set with Sigmoid), mlpT = (0.5*w2T).T @ hh'.  Gate applied with per-partition
tensor_scalar_mul in place, then DMA out.
"""

import numpy as np

import concourse.bacc as bacc
import concourse.bass as bass
import concourse.mybir as mybir
import concourse.tile as tile
from concourse.bass_utils import run_bass_kernel_spmd

B = 32
C = 768
HW = 56 * 56  # 3136
HID = 48      # C // 16
NCORES = 8
B_LOC = B // NCORES  # 4
KC = C // 128        # 6 channel chunks
F32 = mybir.dt.float32
AF = mybir.ActivationFunctionType
ALU = mybir.AluOpType

_cache = {}


def _build_nc():
    nc = bacc.Bacc("TRN2", target_bir_lowering=False, debug=False)
    x_d = nc.declare_dram_parameter("x", [B_LOC * C, HW], F32, isOutput=False)
    # host-pretransposed weights: w1t[p, k, h] = w1[h, k*128+p],
    # w2t[h, k, p] = 0.5 * w2[k*128+p, h]  (0.5 folds the gelu half)
    w1_d = nc.declare_dram_parameter("w1t", [128, KC * HID], F32, isOutput=False)
    w2_d = nc.declare_dram_parameter("w2t", [HID, KC * 128], F32, isOutput=False)
    out_d = nc.declare_dram_parameter("out", [B_LOC * C, HW], F32, isOutput=True)

    with tile.TileContext(nc) as tc:
        with (
            tc.tile_pool(name="consts", bufs=1) as consts,
            tc.tile_pool(name="xtiles", bufs=6) as xpool,
            tc.tile_pool(name="otiles", bufs=9) as opool,
            tc.tile_pool(name="pooled", bufs=3) as pooled_pool,
            tc.tile_pool(name="small", bufs=3) as small_pool,
            tc.tile_pool(name="psum", bufs=2, space="PSUM") as psum_pool,
        ):
            w1T = consts.tile([128, KC, HID], F32)
            nc.sync.dma_start(
                out=w1T, in_=w1_d.rearrange("p (k h) -> p k h", k=KC)
            )
            w2T = consts.tile([HID, KC, 128], F32)
            nc.sync.dma_start(
                out=w2T, in_=w2_d.rearrange("h (k p) -> h k p", k=KC)
            )

            for b in range(B_LOC):
                ots = []
                pooled = pooled_pool.tile([128, KC, 2], F32)
                for k in range(KC):
                    xt = xpool.tile([128, HW], F32, tag="x")
                    ot = opool.tile([128, HW], F32, tag="o")
                    row = (b * KC + k) * 128
                    nc.sync.dma_start(out=xt, in_=x_d[row : row + 128, :])
                    # sum-pool on ACT while staging x into the write buffer;
                    # xt frees right after pooling, decoupled from the gate
                    nc.scalar.activation(
                        out=ot,
                        in_=xt,
                        func=AF.Copy,
                        accum_out=pooled[:, k, 0:1],
                    )
                    nc.vector.reduce_max(
                        out=pooled[:, k, 1:2], in_=xt, axis=mybir.AxisListType.X
                    )
                    ots.append(ot)

                # hT [48, 2] = sum_k w1T_k.T @ pooledT_k
                hps = psum_pool.tile([HID, 2], F32, tag="hps")
                for k in range(KC):
                    nc.tensor.matmul(
                        hps,
                        w1T[:, k, :],
                        pooled[:, k, :],
                        start=(k == 0),
                        stop=(k == KC - 1),
                    )
                # avg column holds the raw sum; scale to the mean here (cheaper
                # than scaling 6 [128,1] pooled slots or a [128,3136] tile)
                nc.vector.tensor_scalar_mul(hps[:, 0:1], hps[:, 0:1], 1.0 / HW)
                e_sb = small_pool.tile([HID, 2], F32, tag="e")
                nc.scalar.activation(
                    out=e_sb, in_=hps, func=AF.Erf, scale=0.7071067811865476
                )
                # hh' = (e + 1) * u   (u = pre-gelu matmul output); the gate
                # path is linear in hh, so accum_out sums avg+max columns
                # directly into hsum for matmul2
                hh = small_pool.tile([HID, 2], F32, tag="hh")
                hsum = small_pool.tile([HID, 1], F32, tag="hsum")
                nc.vector.scalar_tensor_tensor(
                    out=hh, in0=e_sb, scalar=1.0, in1=hps,
                    op0=ALU.add, op1=ALU.mult, accum_out=hsum,
                )
                mlp = psum_pool.tile([128, KC], F32, tag="mlp")
                for k in range(KC):
                    nc.tensor.matmul(
                        mlp[:, k : k + 1],
                        w2T[:, k, :],
                        hsum,
                        start=True,
                        stop=True,
                    )
                gate = small_pool.tile([128, KC], F32, tag="gate")
                nc.scalar.activation(out=gate, in_=mlp, func=AF.Sigmoid)

                for k in range(KC):
                    ot = ots[k]
                    row = (b * KC + k) * 128
                    # split the 6 gate-multiplies across DVE and ACT so the
                    # sample tail isn't serialized on one engine
                    if k % 3 == 2:
                        nc.scalar.activation(
                            out=ot, in_=ot, func=AF.Copy,
                            scale=gate[:, k : k + 1],
                        )
                    else:
                        nc.vector.tensor_scalar_mul(ot, ot, gate[:, k : k + 1])
                    # writes ride SWDGE (GpSimd) so they never head-of-line
                    # block the read FIFO on the Sync HWDGE ring
                    nc.gpsimd.dma_start(out=out_d[row : row + 128, :], in_=ot)
    nc.finalize()
    return nc


def kernel(x, w1, w2, _trace=False):
    if "nc" not in _cache:
        _cache["nc"] = _build_nc()
    nc = _cache["nc"]

    x = np.ascontiguousarray(x, dtype=np.float32)
    w1t = np.ascontiguousarray(
        np.asarray(w1, np.float32).reshape(HID, KC, 128).transpose(2, 1, 0)
        .reshape(128, KC * HID)
    )
    w2t = np.ascontiguousarray(
        (0.5 * np.asarray(w2, np.float32)).reshape(KC, 128, HID)
        .transpose(2, 0, 1).reshape(HID, KC * 128)
    )
    in_maps = [
        {
            "x": x[i * B_LOC : (i + 1) * B_LOC].reshape(B_LOC * C, HW),
            "w1t": w1t,
            "w2t": w2t,
        }
        for i in range(NCORES)
    ]
    res = run_bass_kernel_spmd(nc, in_maps, core_ids=list(range(NCORES)),
                               trace=_trace)
    out = np.concatenate(
        [r["out"].reshape(B_LOC, C, 56, 56) for r in res.results], axis=0
    )
    if _trace:
        _cache["last_results"] = res
    return out
